# revision 25
# baseline (speedup 1.0000x reference)
"""Trainium2 Bass kernel: Longformer block (banded self-attention + FFN).

Full inputs come in, full output goes out. Internally shards over 8
NeuronCores: core c handles batch c//4, sequence chunk c%4 (1024 tokens),
with a 128-token halo on each side for the local attention window.
Everything else (QKV, banded softmax, Wo, LN1, FFN, LN2) is computed
per-core with no collectives.
"""

import sys
import types
import numpy as np
import ml_dtypes
from contextlib import ExitStack

import concourse.bass as bass
import concourse.mybir as mybir
import concourse.tile as tile
from concourse.vector_clock import ScopedClock
from concourse.bass_utils import run_bass_kernel_spmd

F32 = mybir.dt.float32
F32R = mybir.dt.float32r
BF16 = mybir.dt.bfloat16
AF = mybir.ActivationFunctionType
OP = mybir.AluOpType

# problem shape (hardcoded)
B, L, D, H, HD, FF = 2, 4096, 512, 8, 64, 2048
CORES = 8
T = (B * L) // CORES          # 1024 tokens per core
PAD = 128                     # halo
EXT = T + 2 * PAD             # 1280
QBS = T // 128                # 8 query blocks / core
SW = 3 * 128                  # 384-wide key strip per query block
EPS = 1e-5
NEG = -1e30

# ---------------------------------------------------------------------------
# The final Tile drain on this walrus build only supports ONE sync-wait per
# CTRL instruction; stock TileContext funnels every live semaphore into a
# single drain.  Split the waits across a chain of drains instead.
_MAX_DRAIN_WAITS = 1


def _patched_drain_and_barrier(self, tick_clock, wait_clock):
    nc = self.nc
    drain_inst = nc.sync.drain()
    wait_clock.add_sem_waits(
        drain_inst.ins, ScopedClock({None: tick_clock.global_clock})
    )
    mi = drain_inst.ins
    if mi.sync_info is not None:
        waits = list(mi.sync_info.on_wait or [])
        if len(waits) > _MAX_DRAIN_WAITS:
            mi.sync_info = mybir.SyncInfo(
                on_wait=waits[:_MAX_DRAIN_WAITS],
                on_update=list(mi.sync_info.on_update or []),
            )
            # spread remaining waits across engines so they resolve in
            # parallel; the all_engine_barrier below orders everyone.
            engines = [nc.vector, nc.scalar, nc.tensor, nc.gpsimd, nc.sync]
            for i, w in enumerate(waits[_MAX_DRAIN_WAITS:]):
                eng = engines[i % len(engines)]
                n2 = eng.nop()
                n2.ins.sync_info = mybir.SyncInfo(on_wait=[w], on_update=[])
    nc.all_engine_barrier()
    assert self.sems is not None
    popped = nc._tile_sem_poison_stack.pop()
    assert popped is self._sem_poison
    nc.clear_and_free_semaphores(list(self.sems.allocated().values()))
    nc.all_engine_barrier()


tile.TileContext._drain_and_barrier = _patched_drain_and_barrier

_MAX_INST_WAITS = 1
_nop_counter = [0]


def _split_excess_waits(nc, limit=_MAX_INST_WAITS):
    """walrus on this image accepts only `limit` sync-waits per instruction;
    move excess waits onto injected same-engine NoOps placed just before."""
    for f in nc.m.functions:
        for bb in f.blocks:
            out = []
            changed = False
            for inst in bb.instructions:
                si = inst.sync_info
                waits = list(si.on_wait) if (si is not None and si.on_wait) else []
                if len(waits) > limit:
                    changed = True
                    extra = waits[: len(waits) - limit]
                    keep = waits[len(waits) - limit :]
                    for i in range(0, len(extra), max(1, limit)):
                        _nop_counter[0] += 1
                        nop = mybir.InstNoOp(name=f"nopw-{_nop_counter[0]}", ins=[], outs=[])
                        nop.engine = inst.engine
                        nop.sync_info = mybir.SyncInfo(
                            on_wait=extra[i : i + max(1, limit)], on_update=[]
                        )
                        out.append(nop)
                    inst.sync_info = mybir.SyncInfo(
                        on_wait=keep, on_update=list(si.on_update or [])
                    )
                out.append(inst)
            if changed:
                bb.instructions = out


# ---------------------------------------------------------------------------
def _build_program(trivial_ln: bool, trivial_b2: bool):
    nc = bass.Bass("TRN2", target_bir_lowering=False, debug=False)

    dram = {}
    def din(name, shape):
        dram[name] = nc.dram_tensor(name, shape, F32, kind="ExternalInput").ap()
        return dram[name]

    xr = din("xr", [T, D])              # x_core + (bv@Wo + bo), residual input
    def dinr(name, shape):
        dram[name] = nc.dram_tensor(name, shape, F32R, kind="ExternalInput").ap()
        return dram[name]

    x_ext = dinr("x_ext", [EXT, D])
    ident = dinr("ident", [128, 128])

    def dinb(name, shape):
        dram[name] = nc.dram_tensor(name, shape, BF16, kind="ExternalInput").ap()
        return dram[name]

    identb = dinb("identb", [128, 128])
    Wod = dinb("Wo", [D, D])
    W1d = dinb("W1", [D, FF]); W2d = dinb("W2", [FF, D])
    vones = dinb("vones", [128, H, 4])
    masks = dinb("masks", [3, 128, SW])

    Wqd = dinr("Wq", [D, D]); Wkd = dinr("Wk", [D, D]); Wvd = dinr("Wv", [D, D])
    bqd = din("bq", [D]); bkd = din("bk", [D])
    b1d = din("b1", [FF]); b2d = din("b2", [D])
    if not trivial_ln:
        g1d = din("ln1_g", [D]); be1d = din("ln1_b", [D])
        g2d = din("ln2_g", [D]); be2d = din("ln2_b", [D])
    out = nc.dram_tensor("out", [T, D], F32, kind="ExternalOutput").ap()

    with tile.TileContext(nc) as tc:
        with ExitStack() as ctx:
            _build_body(ctx, tc, dram, out, trivial_ln, trivial_b2)
    return nc


def _build_body(ctx, tc, dram, out, trivial_ln, trivial_b2):
    nc = tc.nc

    def pool(name, bufs, space="SBUF"):
        return ctx.enter_context(tc.tile_pool(name=name, bufs=bufs, space=space))

    p_const = pool("const", 1)
    p_mm = pool("mm", 2, "PSUM")
    p_fa = pool("facc", 4, "PSUM")
    p_actT = pool("actT", 4)     # xT chunks, slots later recycled for hT
    p_kT = pool("kT", 4)
    p_v = pool("v", 10)
    p_qT = pool("qT", 4)
    p_xn = pool("xn", 2)
    p_w = pool("w", 6)
    p_wo = pool("wo", 4)
    p_w1 = pool("w1", 12)
    p_w2 = pool("w2", 16)
    p_xr = pool("xr", 2)
    p_probs = pool("probs", 5)
    p_pT = pool("pT", 3)
    p_attn = pool("attn", 2)
    p_aT = pool("aT", 2)
    p_r = pool("r", 8)
    p_ffr = pool("ffr", 3)
    p_r2 = pool("r2", 3)
    p_out = pool("out", 3)
    p_sm = pool("sm", 4)

    # ---- constants
    ident_sb = p_const.tile([128, 128], F32R, name="ident", tag="ident")
    nc.sync.dma_start(ident_sb[:], dram["ident"][:])
    identb_sb = p_const.tile([128, 128], BF16, name="identb", tag="identb")
    nc.sync.dma_start(identb_sb[:], dram["identb"][:])
    maskT_sb = []
    for m in range(3):
        mt = p_const.tile([128, SW], BF16, name=f"maskT{m}", tag=f"maskT{m}")
        maskT_sb.append(mt)
    bq_sb = p_const.tile([128, 4], F32, name="bq", tag="bq")
    nc.sync.dma_start(bq_sb[:], dram["bq"].rearrange("(j p) -> p j", p=128))
    bk_sb = p_const.tile([128, 4], F32, name="bk", tag="bk")
    nc.sync.dma_start(bk_sb[:], dram["bk"].rearrange("(j p) -> p j", p=128))
    b1_sb = p_const.tile([128, 16], F32, name="b1", tag="b1")
    nc.sync.dma_start(b1_sb[:], dram["b1"].rearrange("(j p) -> p j", p=128))
    if not trivial_b2:
        b2b_sb = p_const.tile([128, D], F32, name="b2b", tag="b2b")
        nc.sync.dma_start(b2b_sb[0:1, :], dram["b2"].rearrange("d -> 1 d"))
        nc.gpsimd.partition_broadcast(b2b_sb[:], b2b_sb[0:1, :])
    eps_sb = p_const.tile([128, 1], F32, name="eps", tag="eps")
    nc.vector.memset(eps_sb[:], EPS)
    mv1_all = p_const.tile([128, QBS, 2], F32, name="mv1all", tag="mv1all")

    gb = {}
    if not trivial_ln:
        for nm in ("ln1_g", "ln1_b", "ln2_g", "ln2_b"):
            tile_ = p_const.tile([128, D], F32, name=nm, tag=nm)
            nc.sync.dma_start(
                tile_[0:1, :], dram[nm].rearrange("d -> 1 d")
            )
            nc.gpsimd.partition_broadcast(tile_[:], tile_[0:1, :])
            gb[nm] = tile_

    # ---- phase 1: load x, build xT (transposed activations) -------------
    xT = [p_actT.tile([128, EXT], F32R, name=f"xT{c}", tag="actT") for c in range(4)]
    for tb in range(EXT // 128):
        xn = p_xn.tile([128, D], F32R, name=f"xn{tb}", tag="xn")
        nc.sync.dma_start(xn[:], dram["x_ext"][tb * 128 : (tb + 1) * 128, :])
        ps = p_mm.tile([128, 512], F32R, name=f"xTp{tb}", tag="mm")
        for j in range(4):
            nc.tensor.matmul(
                ps[:, j * 128 : (j + 1) * 128],
                xn[:, j * 128 : (j + 1) * 128],
                ident_sb[:],
                is_transpose=True,
                start=(j == 0),
                stop=(j == 3),
            )
        for j in range(4):
            nc.vector.tensor_copy(
                xT[j][:, tb * 128 : (tb + 1) * 128], ps[:, j * 128 : (j + 1) * 128]
            )

    # ---- phase 2: QKV projections ---------------------------------------
    kT = [p_kT.tile([128, EXT], BF16, name=f"kT{c}", tag="kT") for c in range(4)]
    qT = [p_qT.tile([128, T], BF16, name=f"qT{c}", tag="qT") for c in range(4)]
    V = [p_v.tile([128, H, HD + 4], BF16, name=f"v{tb}", tag="v") for tb in range(EXT // 128)]

    def lin_T(dst, wd, bias_sb, tok0, ntok, wname):
        """dst[j] [128, ntok] = (x @ W + b).T ; x cols [tok0, tok0+ntok) of xT."""
        wt = []
        for c in range(4):
            w = p_w.tile([128, D], F32R, name=f"{wname}{c}", tag="w")
            nc.sync.dma_start(w[:], wd[c * 128 : (c + 1) * 128, :])
            wt.append(w)
        for j in range(4):
            for t0 in range(0, ntok, 512):
                tw = min(512, ntok - t0)
                ps = p_mm.tile([128, 512], F32, name=f"{wname}p{j}_{t0}", tag="mm")
                for c in range(4):
                    nc.tensor.matmul(
                        ps[:, :tw],
                        wt[c][:, j * 128 : (j + 1) * 128],
                        xT[c][:, tok0 + t0 : tok0 + t0 + tw],
                        start=(c == 0),
                        stop=(c == 3),
                    )
                if bias_sb is not None:
                    nc.vector.tensor_scalar_add(
                        dst[j][:, t0 : t0 + tw], ps[:, :tw], bias_sb[:, j : j + 1]
                    )
                else:
                    nc.vector.tensor_copy(dst[j][:, t0 : t0 + tw], ps[:, :tw])

    lin_T(kT, dram["Wk"], bk_sb, 0, EXT, "wk")
    lin_T(qT, dram["Wq"], bq_sb, PAD, T, "wq")

    # V in natural layout [tok, dv]; bv folded into xr on the host.
    wvt = []
    for c in range(4):
        w = p_w.tile([128, D], F32R, name=f"wv{c}", tag="w")
        nc.sync.dma_start(w[:], dram["Wv"][c * 128 : (c + 1) * 128, :])
        wvt.append(w)
    for tb in range(EXT // 128):
        ps = p_mm.tile([128, 512], F32, name=f"vp{tb}", tag="mm")
        for c in range(4):
            nc.tensor.matmul(
                ps[:],
                xT[c][:, tb * 128 : (tb + 1) * 128],
                wvt[c][:],
                start=(c == 0),
                stop=(c == 3),
            )
        nc.vector.tensor_copy(
            V[tb][:, :, 0:HD], ps[:].rearrange("p (h d) -> p h d", d=HD)
        )
        nc.sync.dma_start(V[tb][:, :, HD : HD + 4], dram["vones"][:])

    for m in range(3):
        nc.sync.dma_start(maskT_sb[m][:], dram["masks"][m])
    # Wo, resident through attention
    wot = []
    for j in range(4):
        w = p_wo.tile([128, D], BF16, name=f"wo{j}", tag="wo")
        nc.sync.dma_start(w[:], dram["Wo"][j * 128 : (j + 1) * 128, :])
        wot.append(w)
    # W2 resident for both FFN token chunks
    w2t_all = []
    for ffj in range(FF // 128):
        w = p_w2.tile([128, D], BF16, name=f"w2r_{ffj}", tag="w2")
        nc.sync.dma_start(w[:], dram["W2"][ffj * 128 : (ffj + 1) * 128, :])
        w2t_all.append(w)

    # ---- phase 3: banded attention + Wo + residual + LN1 ----------------
    hT = [p_actT.tile([128, T], BF16, name=f"hT{c}", tag="actT") for c in range(4)]
    h_tiles = []

    def layer_norm(r_t, dst, g_key, qb):
        st6 = p_sm.tile([128, 6], F32, name=f"st{g_key}{qb}", tag="st6")
        nc.vector.bn_stats(st6[:], r_t[:])
        mv = p_sm.tile([128, 2], F32, name=f"mv{g_key}{qb}", tag="mv")
        nc.vector.bn_aggr(mv[:], st6[:])
        std = p_sm.tile([128, 1], F32, name=f"sd{g_key}{qb}", tag="std")
        nc.scalar.activation(std[:], mv[:, 1:2], AF.Sqrt, bias=eps_sb[:])
        rs = p_sm.tile([128, 1], F32, name=f"rs{g_key}{qb}", tag="rs")
        nc.vector.reciprocal(rs[:], std[:])
        nm = p_sm.tile([128, 1], F32, name=f"nm{g_key}{qb}", tag="nm")
        nc.vector.scalar_tensor_tensor(
            nm[:], mv[:, 0:1], -1.0, rs[:], op0=OP.mult, op1=OP.mult
        )
        nc.scalar.activation(dst[:], r_t[:], AF.Identity, bias=nm[:], scale=rs[:])
        if not trivial_ln:
            nc.vector.tensor_mul(dst[:], dst[:], gb[f"{g_key}_g"][:])
            nc.vector.tensor_add(dst[:], dst[:], gb[f"{g_key}_b"][:])

    r_tiles = []

    def _ln1_flush(lo, hi):
        """LN1 sqrt/recip/apply + h transposes for q-blocks [lo, hi)."""
        std4 = p_sm.tile([128, hi - lo], F32, name=f"std{lo}", tag="std8")
        nc.scalar.activation(std4[:], mv1_all[:, lo:hi, 1], AF.Sqrt, bias=eps_sb[:])
        rec4 = p_sm.tile([128, hi - lo], F32, name=f"recb{lo}", tag="rec8")
        nc.vector.reciprocal(rec4[:], std4[:])
        nm4 = p_sm.tile([128, hi - lo], F32, name=f"nmb{lo}", tag="nm8")
        nc.vector.scalar_tensor_tensor(
            nm4[:], mv1_all[:, lo:hi, 0], -1.0, rec4[:], op0=OP.mult, op1=OP.mult
        )
        for qb2 in range(lo, hi):
            h_t = r_tiles[qb2]
            nc.scalar.activation(
                h_t[:], r_tiles[qb2][:], AF.Identity,
                bias=nm4[:, qb2 - lo : qb2 - lo + 1],
                scale=rec4[:, qb2 - lo : qb2 - lo + 1],
            )
            if not trivial_ln:
                nc.vector.tensor_mul(h_t[:], h_t[:], gb["ln1_g"][:])
                nc.vector.tensor_add(h_t[:], h_t[:], gb["ln1_b"][:])
            h_tiles.append(h_t)
            h_bf = p_aT.tile([128, D], BF16, name=f"hbf{qb2}", tag="hbf", bufs=2)
            nc.vector.tensor_copy(h_bf[:], h_t[:])
            hT_ps = p_mm.tile([128, 1024], BF16, name=f"hTp{qb2}", tag="mmb", bufs=2, padded_shape=[128, 1024])
            for j in range(4):
                nc.tensor.matmul(
                    hT_ps[:, j * 128 : (j + 1) * 128],
                    h_bf[:, j * 128 : (j + 1) * 128],
                    identb_sb[:],
                    is_transpose=True,
                    start=(j == 0),
                    stop=(j == 3),
                )
            for j in range(4):
                nc.vector.tensor_copy(
                    hT[j][:, qb2 * 128 : (qb2 + 1) * 128],
                    hT_ps[:, j * 128 : (j + 1) * 128],
                )

    for qb in range(QBS):
        xr_t = p_xr.tile([128, D], F32, name=f"xr{qb}", tag="xr")
        nc.sync.dma_start(xr_t[:], dram["xr"][qb * 128 : (qb + 1) * 128, :])
        attn_sb = p_attn.tile([128, D], BF16, name=f"attn{qb}", tag="attn")
        rec = p_sm.tile([128, H], F32, name=f"rec{qb}", tag="rec")
        mk = maskT_sb[0] if qb == 0 else (
            maskT_sb[2] if qb == QBS - 1 else maskT_sb[1])

        sc_ps = [None] * H

        def emit_scores(h):
            cch, roff = h // 2, (h % 2) * 64
            ps = p_fa.tile([128, 512], F32, name=f"sc{qb}_{h}", tag="facc")
            nc.tensor.matmul(
                ps[:, :SW],
                qT[cch][roff : roff + 64, qb * 128 : (qb + 1) * 128],
                kT[cch][roff : roff + 64, qb * 128 : qb * 128 + SW],
                start=True,
                stop=True,
            )
            sc_ps[h] = ps

        for h0 in range(4):
            emit_scores(h0)

        prs = [None] * H
        pT_pss = [None] * H
        pTs = [None] * H

        def emit_exp(h):
            pr = p_probs.tile([128, SW], BF16, name=f"pr{qb}_{h}", tag="probs")
            nc.scalar.activation(
                pr[:], sc_ps[h][:, :SW], AF.Exp, scale=float(1.0 / np.sqrt(HD))
            )
            prs[h] = pr

        def emit_T(h):
            pT_ps = p_mm.tile([128, 1024], BF16, name=f"pT{qb}_{h}", tag="mmb", bufs=2, padded_shape=[128, 1024])
            for kb in range(3):
                nc.tensor.matmul(
                    pT_ps[:, kb * 128 : (kb + 1) * 128],
                    prs[h][:, kb * 128 : (kb + 1) * 128],
                    identb_sb[:],
                    is_transpose=True,
                    start=(kb == 0),
                    stop=(kb == 2),
                )
            pT_pss[h] = pT_ps

        emit_exp(0)
        emit_exp(1)
        emit_T(0)
        for h in range(H):
            if h + 2 < H:
                emit_exp(h + 2)
            if h + 1 < H:
                emit_T(h + 1)
            pT = p_pT.tile([128, SW], BF16, name=f"pTs{qb}_{h}", tag="pT")
            nc.vector.tensor_tensor(pT[:], pT_pss[h][:, :SW], mk[:], op=OP.mult)
            if h + 4 < H:
                emit_scores(h + 4)
            at_ps = p_mm.tile([128, 512], F32, name=f"at{qb}_{h}", tag="mm")
            for kb in range(3):
                nc.tensor.matmul(
                    at_ps[:, : HD + 4],
                    pT[:, kb * 128 : (kb + 1) * 128],
                    V[qb + kb][:, h, :],
                    start=(kb == 0),
                    stop=(kb == 2),
                )
            nc.vector.reciprocal(rec[:, h : h + 1], at_ps[:, HD : HD + 1])
            nc.vector.tensor_scalar_mul(
                attn_sb[:, h * HD : (h + 1) * HD], at_ps[:, :HD], rec[:, h : h + 1]
            )
        # attn -> attn_T -> @Wo -> + xr ; LN1 stats only (apply deferred)
        aT_ps = p_mm.tile([128, 1024], BF16, name=f"aTp{qb}", tag="mmb", bufs=2, padded_shape=[128, 1024])
        for j in range(4):
            nc.tensor.matmul(
                aT_ps[:, j * 128 : (j + 1) * 128],
                attn_sb[:, j * 128 : (j + 1) * 128],
                identb_sb[:],
                is_transpose=True,
                start=(j == 0),
                stop=(j == 3),
            )
        aT = p_aT.tile([128, D], BF16, name=f"aT{qb}", tag="aT")
        nc.vector.tensor_copy(aT[:], aT_ps[:, :D])
        wo_ps = p_mm.tile([128, 512], F32, name=f"wop{qb}", tag="mm")
        for j in range(4):
            nc.tensor.matmul(
                wo_ps[:],
                aT[:, j * 128 : (j + 1) * 128],
                wot[j][:],
                start=(j == 0),
                stop=(j == 3),
            )
        r_t = p_r.tile([128, D], F32R, name=f"r{qb}", tag="r")
        nc.vector.tensor_tensor(r_t[:], wo_ps[:], xr_t[:], op=OP.add)
        r_tiles.append(r_t)
        st6 = p_sm.tile([128, 6], F32, name=f"st1_{qb}", tag="st6")
        nc.vector.bn_stats(st6[:], r_t[:])
        nc.vector.bn_aggr(mv1_all[:, qb, :], st6[:])
        if qb == QBS - 1:
            _ln1_flush(0, QBS)

    # (deferred LN1 halves are emitted inside the qb loop; see _ln1_flush)

    # ---- phase 4: FFN + residual + LN2 ----------------------------------
    # FFN2 emits the output in natural [tok, D] layout directly:
    # fa[tb] += rr[ffj][:, tb-block].T @ W2[ffj]  (accumulated over ffj)
    for t in range(2):
        fa = [
            p_fa.tile([128, 512], F32, name=f"fa{t}_{j}", tag="facc") for j in range(4)
        ]
        for ffj in range(FF // 128):
            w1t = p_w1.tile([128, 4, 128], BF16, name=f"w1_{t}_{ffj}", tag="w1")
            nc.sync.dma_start(
                w1t[:],
                dram["W1"].rearrange("(c p) (f n) -> f p c n", p=128, n=128)[ffj],
            )
            w2t = w2t_all[ffj]
            f1_ps = p_mm.tile([128, 512], F32, name=f"f1_{t}_{ffj}", tag="mm")
            for c in range(4):
                nc.tensor.matmul(
                    f1_ps[:],
                    w1t[:, c, :],
                    hT[c][:, t * 512 : (t + 1) * 512],
                    start=(c == 0),
                    stop=(c == 3),
                )
            rr = p_ffr.tile([128, 512], BF16, name=f"rr{t}_{ffj}", tag="ffr")
            nc.scalar.activation(
                rr[:], f1_ps[:], AF.Relu, bias=b1_sb[:, ffj : ffj + 1]
            )
            for tb in range(4):
                nc.tensor.matmul(
                    fa[tb][:],
                    rr[:, tb * 128 : (tb + 1) * 128],
                    w2t[:],
                    start=(ffj == 0),
                    stop=(ffj == FF // 128 - 1),
                )
        for tb in range(4):
            qb = t * 4 + tb
            r2 = p_r2.tile([128, D], F32, name=f"r2_{qb}", tag="r2")
            nc.vector.tensor_tensor(r2[:], fa[tb][:], h_tiles[qb][:], op=OP.add)
            if not trivial_b2:
                nc.vector.tensor_add(r2[:], r2[:], b2b_sb[:])
            o_t = p_out.tile([128, D], F32, name=f"o{qb}", tag="out")
            layer_norm(r2, o_t, "ln2", qb)
            nc.sync.dma_start(out[qb * 128 : (qb + 1) * 128, :], o_t[:])


# ---------------------------------------------------------------------------
_PROG_CACHE = {}


def _get_program(trivial_ln: bool, trivial_b2: bool = True):
    key = (trivial_ln, trivial_b2)
    if key not in _PROG_CACHE:
        _PROG_CACHE[key] = _build_program(trivial_ln, trivial_b2)
    return _PROG_CACHE[key]


def _host_masks(window: int, chunk: int):
    """Multiplicative masks in block-transposed layout: tile[kk, kb*128+qq] =
    1.0 iff key (kb*128+kk) is visible to query qq of the block."""
    qq = np.arange(128)[:, None]
    kk = np.arange(SW)[None, :]
    band = (np.abs(kk - qq - PAD) <= window).astype(np.float32)
    m_first = band.copy()
    m_last = band.copy()
    if chunk == 0:
        m_first[:, :PAD] = 0.0
    if chunk == (L // T) - 1:
        m_last[:, SW - PAD :] = 0.0

    def blockT(m):
        out = np.empty((128, SW), np.float32)
        for kb in range(3):
            out[:, kb * 128 : (kb + 1) * 128] = m[:, kb * 128 : (kb + 1) * 128].T
        return out

    return np.stack([blockT(m_first), blockT(band), blockT(m_last)]).astype(
        ml_dtypes.bfloat16
    )


def kernel(**inputs):
    x = np.ascontiguousarray(np.asarray(inputs["x"], dtype=np.float32))
    window = int(np.asarray(inputs["window"]))
    assert 0 <= window <= PAD, f"window {window} > {PAD} not supported"
    Wq = np.ascontiguousarray(np.asarray(inputs["Wq"], np.float32))
    Wk = np.ascontiguousarray(np.asarray(inputs["Wk"], np.float32))
    Wv = np.ascontiguousarray(np.asarray(inputs["Wv"], np.float32))
    Wo = np.ascontiguousarray(np.asarray(inputs["Wo"], np.float32))
    W1 = np.ascontiguousarray(np.asarray(inputs["W1"], np.float32))
    W2 = np.ascontiguousarray(np.asarray(inputs["W2"], np.float32))
    bq = np.asarray(inputs["bq"], np.float32)
    bk = np.asarray(inputs["bk"], np.float32)
    bv = np.asarray(inputs["bv"], np.float32)
    bo = np.asarray(inputs["bo"], np.float32)
    b1 = np.asarray(inputs["b1"], np.float32)
    b2 = np.asarray(inputs["b2"], np.float32)
    g1 = np.asarray(inputs["ln1_g"], np.float32)
    be1 = np.asarray(inputs["ln1_b"], np.float32)
    g2 = np.asarray(inputs["ln2_g"], np.float32)
    be2 = np.asarray(inputs["ln2_b"], np.float32)

    trivial_ln = (
        np.all(g1 == 1.0) and np.all(be1 == 0.0)
        and np.all(g2 == 1.0) and np.all(be2 == 0.0)
    )
    trivial_b2 = bool(np.all(b2 == 0.0))
    nc = _get_program(bool(trivial_ln), trivial_b2)

    bo_p = (bv @ Wo + bo).astype(np.float32)
    ident = np.eye(128, dtype=np.float32)
    chunks = L // T  # seq chunks per batch element (4)

    in_maps = []
    for c in range(CORES):
        b, j = divmod(c, chunks)
        s = j * T
        xe = np.zeros((EXT, D), np.float32)
        lo, hi = max(0, s - PAD), min(L, s + T + PAD)
        xe[lo - (s - PAD) : hi - (s - PAD)] = x[b, lo:hi]
        vones = np.zeros((128, H, 4), ml_dtypes.bfloat16)
        vones[:, :, 0] = 1.0
        m = {
            "x_ext": xe,
            "vones": vones,
            "identb": np.eye(128, dtype=ml_dtypes.bfloat16),
            "xr": (x[b, s : s + T] + bo_p).astype(np.float32),
            "masks": _host_masks(window, j),
            "ident": ident,
            "Wq": Wq, "Wk": Wk, "Wv": Wv,
            "Wo": Wo.astype(ml_dtypes.bfloat16),
            "W1": W1.astype(ml_dtypes.bfloat16),
            "W2": W2.astype(ml_dtypes.bfloat16),
            "bq": bq, "bk": bk, "b1": b1, "b2": b2,
        }
        if not trivial_ln:
            m.update({"ln1_g": g1, "ln1_b": be1, "ln2_g": g2, "ln2_b": be2})
        in_maps.append(m)

    if not getattr(nc, "_waits_split", False):
        _split_excess_waits(nc)
        nc._waits_split = True
    res = run_bass_kernel_spmd(nc, in_maps, core_ids=list(range(CORES)))
    outs = np.stack([res.results[c]["out"] for c in range(CORES)])
    return outs.reshape(B, L, D)


# revision 26
# speedup vs baseline: 1.0238x; 1.0238x over previous
"""Trainium2 Bass kernel: Longformer block (banded self-attention + FFN).

Full inputs come in, full output goes out. Internally shards over 8
NeuronCores: core c handles batch c//4, sequence chunk c%4 (1024 tokens),
with a 128-token halo on each side for the local attention window.
Everything else (QKV, banded softmax, Wo, LN1, FFN, LN2) is computed
per-core with no collectives.
"""

import sys
import types
import numpy as np
import ml_dtypes
from contextlib import ExitStack

import concourse.bass as bass
import concourse.mybir as mybir
import concourse.tile as tile
from concourse.vector_clock import ScopedClock
from concourse.bass_utils import run_bass_kernel_spmd

F32 = mybir.dt.float32
F32R = mybir.dt.float32r
BF16 = mybir.dt.bfloat16
AF = mybir.ActivationFunctionType
OP = mybir.AluOpType

# problem shape (hardcoded)
B, L, D, H, HD, FF = 2, 4096, 512, 8, 64, 2048
CORES = 8
T = (B * L) // CORES          # 1024 tokens per core
PAD = 128                     # halo
EXT = T + 2 * PAD             # 1280
QBS = T // 128                # 8 query blocks / core
SW = 3 * 128                  # 384-wide key strip per query block
EPS = 1e-5
NEG = -1e30

# ---------------------------------------------------------------------------
# The final Tile drain on this walrus build only supports ONE sync-wait per
# CTRL instruction; stock TileContext funnels every live semaphore into a
# single drain.  Split the waits across a chain of drains instead.
_MAX_DRAIN_WAITS = 1


def _patched_drain_and_barrier(self, tick_clock, wait_clock):
    nc = self.nc
    drain_inst = nc.sync.drain()
    wait_clock.add_sem_waits(
        drain_inst.ins, ScopedClock({None: tick_clock.global_clock})
    )
    mi = drain_inst.ins
    if mi.sync_info is not None:
        waits = list(mi.sync_info.on_wait or [])
        if len(waits) > _MAX_DRAIN_WAITS:
            mi.sync_info = mybir.SyncInfo(
                on_wait=waits[:_MAX_DRAIN_WAITS],
                on_update=list(mi.sync_info.on_update or []),
            )
            # spread remaining waits across engines so they resolve in
            # parallel; the all_engine_barrier below orders everyone.
            engines = [nc.vector, nc.scalar, nc.tensor, nc.gpsimd, nc.sync]
            for i, w in enumerate(waits[_MAX_DRAIN_WAITS:]):
                eng = engines[i % len(engines)]
                n2 = eng.nop()
                n2.ins.sync_info = mybir.SyncInfo(on_wait=[w], on_update=[])
    nc.all_engine_barrier()
    assert self.sems is not None
    popped = nc._tile_sem_poison_stack.pop()
    assert popped is self._sem_poison
    nc.clear_and_free_semaphores(list(self.sems.allocated().values()))
    nc.all_engine_barrier()


tile.TileContext._drain_and_barrier = _patched_drain_and_barrier

_MAX_INST_WAITS = 1
_nop_counter = [0]


def _split_excess_waits(nc, limit=_MAX_INST_WAITS):
    """walrus on this image accepts only `limit` sync-waits per instruction;
    move excess waits onto injected same-engine NoOps placed just before."""
    for f in nc.m.functions:
        for bb in f.blocks:
            out = []
            changed = False
            for inst in bb.instructions:
                si = inst.sync_info
                waits = list(si.on_wait) if (si is not None and si.on_wait) else []
                if len(waits) > limit:
                    changed = True
                    extra = waits[: len(waits) - limit]
                    keep = waits[len(waits) - limit :]
                    for i in range(0, len(extra), max(1, limit)):
                        _nop_counter[0] += 1
                        nop = mybir.InstNoOp(name=f"nopw-{_nop_counter[0]}", ins=[], outs=[])
                        nop.engine = inst.engine
                        nop.sync_info = mybir.SyncInfo(
                            on_wait=extra[i : i + max(1, limit)], on_update=[]
                        )
                        out.append(nop)
                    inst.sync_info = mybir.SyncInfo(
                        on_wait=keep, on_update=list(si.on_update or [])
                    )
                out.append(inst)
            if changed:
                bb.instructions = out


# ---------------------------------------------------------------------------
def _build_program(trivial_ln: bool, trivial_b2: bool):
    nc = bass.Bass("TRN2", target_bir_lowering=False, debug=False)

    dram = {}
    def din(name, shape):
        dram[name] = nc.dram_tensor(name, shape, F32, kind="ExternalInput").ap()
        return dram[name]

    xr = din("xr", [T, D])              # x_core + (bv@Wo + bo), residual input
    def dinr(name, shape):
        dram[name] = nc.dram_tensor(name, shape, F32R, kind="ExternalInput").ap()
        return dram[name]

    x_ext = dinr("x_ext", [EXT, D])
    ident = dinr("ident", [128, 128])

    def dinb(name, shape):
        dram[name] = nc.dram_tensor(name, shape, BF16, kind="ExternalInput").ap()
        return dram[name]

    identb = dinb("identb", [128, 128])
    Wod = dinb("Wo", [D, D])
    W1d = dinb("W1", [D, FF]); W2d = dinb("W2", [FF, D])
    vones = dinb("vones", [128, H, 4])
    masks = dinb("masks", [3, 128, SW])

    Wqd = dinr("Wq", [D, D]); Wkd = dinr("Wk", [D, D]); Wvd = dinr("Wv", [D, D])
    bqd = din("bq", [D]); bkd = din("bk", [D])
    b1d = din("b1", [FF]); b2d = din("b2", [D])
    if not trivial_ln:
        g1d = din("ln1_g", [D]); be1d = din("ln1_b", [D])
        g2d = din("ln2_g", [D]); be2d = din("ln2_b", [D])
    out = nc.dram_tensor("out", [T, D], F32, kind="ExternalOutput").ap()

    with tile.TileContext(nc) as tc:
        with ExitStack() as ctx:
            _build_body(ctx, tc, dram, out, trivial_ln, trivial_b2)
    return nc


def _build_body(ctx, tc, dram, out, trivial_ln, trivial_b2):
    nc = tc.nc

    def pool(name, bufs, space="SBUF"):
        return ctx.enter_context(tc.tile_pool(name=name, bufs=bufs, space=space))

    p_const = pool("const", 1)
    p_mm = pool("mm", 2, "PSUM")
    p_fa = pool("facc", 4, "PSUM")
    p_actT = pool("actT", 4)     # xT chunks, slots later recycled for hT
    p_kT = pool("kT", 4)
    p_v = pool("v", 10)
    p_qT = pool("qT", 4)
    p_xn = pool("xn", 2)
    p_w = pool("w", 6)
    p_wo = pool("wo", 4)
    p_w1 = pool("w1", 10)
    p_w2 = pool("w2", 16)
    p_xr = pool("xr", 2)
    p_probs = pool("probs", 5)
    p_pT = pool("pT", 3)
    p_attn = pool("attn", 2)
    p_aT = pool("aT", 2)
    p_r = pool("r", 8)
    p_ffr = pool("ffr", 3)
    p_r2 = pool("r2", 3)
    p_out = pool("out", 3)
    p_sm = pool("sm", 4)

    # ---- constants
    ident_sb = p_const.tile([128, 128], F32R, name="ident", tag="ident")
    nc.sync.dma_start(ident_sb[:], dram["ident"][:])
    identb_sb = p_const.tile([128, 128], BF16, name="identb", tag="identb")
    nc.sync.dma_start(identb_sb[:], dram["identb"][:])
    maskT_sb = []
    for m in range(3):
        mt = p_const.tile([128, SW], BF16, name=f"maskT{m}", tag=f"maskT{m}")
        maskT_sb.append(mt)
    bq_sb = p_const.tile([128, 4], F32, name="bq", tag="bq")
    nc.sync.dma_start(bq_sb[:], dram["bq"].rearrange("(j p) -> p j", p=128))
    bk_sb = p_const.tile([128, 4], F32, name="bk", tag="bk")
    nc.sync.dma_start(bk_sb[:], dram["bk"].rearrange("(j p) -> p j", p=128))
    b1_sb = p_const.tile([128, 16], F32, name="b1", tag="b1")
    nc.sync.dma_start(b1_sb[:], dram["b1"].rearrange("(j p) -> p j", p=128))
    if not trivial_b2:
        b2b_sb = p_const.tile([128, D], F32, name="b2b", tag="b2b")
        nc.sync.dma_start(b2b_sb[0:1, :], dram["b2"].rearrange("d -> 1 d"))
        nc.gpsimd.partition_broadcast(b2b_sb[:], b2b_sb[0:1, :])
    eps_sb = p_const.tile([128, 1], F32, name="eps", tag="eps")
    nc.vector.memset(eps_sb[:], EPS)
    mv1_all = p_const.tile([128, QBS, 2], F32, name="mv1all", tag="mv1all")

    gb = {}
    if not trivial_ln:
        for nm in ("ln1_g", "ln1_b", "ln2_g", "ln2_b"):
            tile_ = p_const.tile([128, D], F32, name=nm, tag=nm)
            nc.sync.dma_start(
                tile_[0:1, :], dram[nm].rearrange("d -> 1 d")
            )
            nc.gpsimd.partition_broadcast(tile_[:], tile_[0:1, :])
            gb[nm] = tile_

    # ---- phase 1: load x, build xT (transposed activations) -------------
    xT = [p_actT.tile([128, EXT], F32R, name=f"xT{c}", tag="actT") for c in range(4)]
    for tb in range(EXT // 128):
        xn = p_xn.tile([128, D], F32R, name=f"xn{tb}", tag="xn")
        nc.sync.dma_start(xn[:], dram["x_ext"][tb * 128 : (tb + 1) * 128, :])
        ps = p_mm.tile([128, 512], F32R, name=f"xTp{tb}", tag="mm")
        for j in range(4):
            nc.tensor.matmul(
                ps[:, j * 128 : (j + 1) * 128],
                xn[:, j * 128 : (j + 1) * 128],
                ident_sb[:],
                is_transpose=True,
                start=(j == 0),
                stop=(j == 3),
            )
        for j in range(4):
            nc.vector.tensor_copy(
                xT[j][:, tb * 128 : (tb + 1) * 128], ps[:, j * 128 : (j + 1) * 128]
            )

    # ---- phase 2: QKV projections ---------------------------------------
    kT = [p_kT.tile([128, EXT], BF16, name=f"kT{c}", tag="kT") for c in range(4)]
    qT = [p_qT.tile([128, T], BF16, name=f"qT{c}", tag="qT") for c in range(4)]
    V = [p_v.tile([128, H, HD + 4], BF16, name=f"v{tb}", tag="v") for tb in range(EXT // 128)]

    def lin_T(dst, wd, bias_sb, tok0, ntok, wname):
        """dst[j] [128, ntok] = (x @ W + b).T ; x cols [tok0, tok0+ntok) of xT."""
        wt = []
        for c in range(4):
            w = p_w.tile([128, D], F32R, name=f"{wname}{c}", tag="w")
            nc.sync.dma_start(w[:], wd[c * 128 : (c + 1) * 128, :])
            wt.append(w)
        for j in range(4):
            for t0 in range(0, ntok, 512):
                tw = min(512, ntok - t0)
                ps = p_mm.tile([128, 512], F32, name=f"{wname}p{j}_{t0}", tag="mm")
                for c in range(4):
                    nc.tensor.matmul(
                        ps[:, :tw],
                        wt[c][:, j * 128 : (j + 1) * 128],
                        xT[c][:, tok0 + t0 : tok0 + t0 + tw],
                        start=(c == 0),
                        stop=(c == 3),
                    )
                if bias_sb is not None:
                    nc.vector.tensor_scalar_add(
                        dst[j][:, t0 : t0 + tw], ps[:, :tw], bias_sb[:, j : j + 1]
                    )
                else:
                    nc.vector.tensor_copy(dst[j][:, t0 : t0 + tw], ps[:, :tw])

    lin_T(kT, dram["Wk"], bk_sb, 0, EXT, "wk")
    lin_T(qT, dram["Wq"], bq_sb, PAD, T, "wq")

    # V in natural layout [tok, dv]; bv folded into xr on the host.
    wvt = []
    for c in range(4):
        w = p_w.tile([128, D], F32R, name=f"wv{c}", tag="w")
        nc.sync.dma_start(w[:], dram["Wv"][c * 128 : (c + 1) * 128, :])
        wvt.append(w)
    for tb in range(EXT // 128):
        ps = p_mm.tile([128, 512], F32, name=f"vp{tb}", tag="mm")
        for c in range(4):
            nc.tensor.matmul(
                ps[:],
                xT[c][:, tb * 128 : (tb + 1) * 128],
                wvt[c][:],
                start=(c == 0),
                stop=(c == 3),
            )
        nc.vector.tensor_copy(
            V[tb][:, :, 0:HD], ps[:].rearrange("p (h d) -> p h d", d=HD)
        )
        nc.sync.dma_start(V[tb][:, :, HD : HD + 4], dram["vones"][:])

    for m in range(3):
        nc.sync.dma_start(maskT_sb[m][:], dram["masks"][m])
    # Wo, resident through attention
    wot = []
    for j in range(4):
        w = p_wo.tile([128, D], BF16, name=f"wo{j}", tag="wo")
        nc.sync.dma_start(w[:], dram["Wo"][j * 128 : (j + 1) * 128, :])
        wot.append(w)
    # W2 resident for both FFN token chunks
    w2t_all = []
    for ffj in range(FF // 128):
        w = p_w2.tile([128, D], BF16, name=f"w2r_{ffj}", tag="w2")
        nc.sync.dma_start(w[:], dram["W2"][ffj * 128 : (ffj + 1) * 128, :])
        w2t_all.append(w)

    # ---- phase 3: banded attention + Wo + residual + LN1 ----------------
    hT = [p_actT.tile([128, T], BF16, name=f"hT{c}", tag="actT") for c in range(4)]
    h_tiles = []

    def layer_norm(r_t, dst, g_key, qb):
        st6 = p_sm.tile([128, 6], F32, name=f"st{g_key}{qb}", tag="st6")
        nc.vector.bn_stats(st6[:], r_t[:])
        mv = p_sm.tile([128, 2], F32, name=f"mv{g_key}{qb}", tag="mv")
        nc.vector.bn_aggr(mv[:], st6[:])
        std = p_sm.tile([128, 1], F32, name=f"sd{g_key}{qb}", tag="std")
        nc.scalar.activation(std[:], mv[:, 1:2], AF.Sqrt, bias=eps_sb[:])
        rs = p_sm.tile([128, 1], F32, name=f"rs{g_key}{qb}", tag="rs")
        nc.vector.reciprocal(rs[:], std[:])
        nm = p_sm.tile([128, 1], F32, name=f"nm{g_key}{qb}", tag="nm")
        nc.vector.scalar_tensor_tensor(
            nm[:], mv[:, 0:1], -1.0, rs[:], op0=OP.mult, op1=OP.mult
        )
        nc.scalar.activation(dst[:], r_t[:], AF.Identity, bias=nm[:], scale=rs[:])
        if not trivial_ln:
            nc.vector.tensor_mul(dst[:], dst[:], gb[f"{g_key}_g"][:])
            nc.vector.tensor_add(dst[:], dst[:], gb[f"{g_key}_b"][:])

    r_tiles = []

    def _ln1_flush(lo, hi):
        """LN1 sqrt/recip/apply + h transposes for q-blocks [lo, hi)."""
        std4 = p_sm.tile([128, hi - lo], F32, name=f"std{lo}", tag="std8")
        nc.scalar.activation(std4[:], mv1_all[:, lo:hi, 1], AF.Sqrt, bias=eps_sb[:])
        rec4 = p_sm.tile([128, hi - lo], F32, name=f"recb{lo}", tag="rec8")
        nc.vector.reciprocal(rec4[:], std4[:])
        nm4 = p_sm.tile([128, hi - lo], F32, name=f"nmb{lo}", tag="nm8")
        nc.vector.scalar_tensor_tensor(
            nm4[:], mv1_all[:, lo:hi, 0], -1.0, rec4[:], op0=OP.mult, op1=OP.mult
        )
        for qb2 in range(lo, hi):
            h_t = r_tiles[qb2]
            nc.scalar.activation(
                h_t[:], r_tiles[qb2][:], AF.Identity,
                bias=nm4[:, qb2 - lo : qb2 - lo + 1],
                scale=rec4[:, qb2 - lo : qb2 - lo + 1],
            )
            if not trivial_ln:
                nc.vector.tensor_mul(h_t[:], h_t[:], gb["ln1_g"][:])
                nc.vector.tensor_add(h_t[:], h_t[:], gb["ln1_b"][:])
            h_tiles.append(h_t)
            h_bf = p_aT.tile([128, D], BF16, name=f"hbf{qb2}", tag="hbf", bufs=2)
            nc.vector.tensor_copy(h_bf[:], h_t[:])
            hT_ps = p_mm.tile([128, 1024], BF16, name=f"hTp{qb2}", tag="mmb", bufs=2, padded_shape=[128, 1024])
            for j in range(4):
                nc.tensor.matmul(
                    hT_ps[:, j * 128 : (j + 1) * 128],
                    h_bf[:, j * 128 : (j + 1) * 128],
                    identb_sb[:],
                    is_transpose=True,
                    start=(j == 0),
                    stop=(j == 3),
                )
            for j in range(4):
                nc.vector.tensor_copy(
                    hT[j][:, qb2 * 128 : (qb2 + 1) * 128],
                    hT_ps[:, j * 128 : (j + 1) * 128],
                )

    for qb in range(QBS):
        xr_t = p_xr.tile([128, D], F32, name=f"xr{qb}", tag="xr")
        nc.sync.dma_start(xr_t[:], dram["xr"][qb * 128 : (qb + 1) * 128, :])
        attn_sb = p_attn.tile([128, D], BF16, name=f"attn{qb}", tag="attn")
        rec = p_sm.tile([128, H], F32, name=f"rec{qb}", tag="rec")
        mk = maskT_sb[0] if qb == 0 else (
            maskT_sb[2] if qb == QBS - 1 else maskT_sb[1])

        sc_ps = [None] * H

        def emit_scores(h):
            cch, roff = h // 2, (h % 2) * 64
            ps = p_fa.tile([128, 512], F32, name=f"sc{qb}_{h}", tag="facc")
            nc.tensor.matmul(
                ps[:, :SW],
                qT[cch][roff : roff + 64, qb * 128 : (qb + 1) * 128],
                kT[cch][roff : roff + 64, qb * 128 : qb * 128 + SW],
                start=True,
                stop=True,
            )
            sc_ps[h] = ps

        for h0 in range(4):
            emit_scores(h0)

        prs = [None] * H
        pT_pss = [None] * H
        pTs = [None] * H

        def emit_exp(h):
            pr = p_probs.tile([128, SW], BF16, name=f"pr{qb}_{h}", tag="probs")
            nc.scalar.activation(
                pr[:], sc_ps[h][:, :SW], AF.Exp, scale=float(1.0 / np.sqrt(HD))
            )
            prs[h] = pr

        def emit_T(h):
            pT_ps = p_mm.tile([128, 1024], BF16, name=f"pT{qb}_{h}", tag="mmb", bufs=2, padded_shape=[128, 1024])
            for kb in range(3):
                nc.tensor.matmul(
                    pT_ps[:, kb * 128 : (kb + 1) * 128],
                    prs[h][:, kb * 128 : (kb + 1) * 128],
                    identb_sb[:],
                    is_transpose=True,
                    start=(kb == 0),
                    stop=(kb == 2),
                )
            pT_pss[h] = pT_ps

        emit_exp(0)
        emit_exp(1)
        emit_T(0)
        for h in range(H):
            if h + 2 < H:
                emit_exp(h + 2)
            if h + 1 < H:
                emit_T(h + 1)
            pT = p_pT.tile([128, SW], BF16, name=f"pTs{qb}_{h}", tag="pT")
            nc.vector.tensor_tensor(pT[:], pT_pss[h][:, :SW], mk[:], op=OP.mult)
            if h + 4 < H:
                emit_scores(h + 4)
            at_ps = p_mm.tile([128, 512], F32, name=f"at{qb}_{h}", tag="mm")
            for kb in range(3):
                nc.tensor.matmul(
                    at_ps[:, : HD + 4],
                    pT[:, kb * 128 : (kb + 1) * 128],
                    V[qb + kb][:, h, :],
                    start=(kb == 0),
                    stop=(kb == 2),
                )
            nc.vector.reciprocal(rec[:, h : h + 1], at_ps[:, HD : HD + 1])
            nc.vector.tensor_scalar_mul(
                attn_sb[:, h * HD : (h + 1) * HD], at_ps[:, :HD], rec[:, h : h + 1]
            )
        # attn -> attn_T -> @Wo -> + xr ; LN1 stats only (apply deferred)
        aT_ps = p_mm.tile([128, 1024], BF16, name=f"aTp{qb}", tag="mmb", bufs=2, padded_shape=[128, 1024])
        for j in range(4):
            nc.tensor.matmul(
                aT_ps[:, j * 128 : (j + 1) * 128],
                attn_sb[:, j * 128 : (j + 1) * 128],
                identb_sb[:],
                is_transpose=True,
                start=(j == 0),
                stop=(j == 3),
            )
        aT = p_aT.tile([128, D], BF16, name=f"aT{qb}", tag="aT")
        nc.vector.tensor_copy(aT[:], aT_ps[:, :D])
        wo_ps = p_mm.tile([128, 512], F32, name=f"wop{qb}", tag="mm")
        for j in range(4):
            nc.tensor.matmul(
                wo_ps[:],
                aT[:, j * 128 : (j + 1) * 128],
                wot[j][:],
                start=(j == 0),
                stop=(j == 3),
            )
        r_t = p_r.tile([128, D], F32R, name=f"r{qb}", tag="r")
        nc.vector.tensor_tensor(r_t[:], wo_ps[:], xr_t[:], op=OP.add)
        r_tiles.append(r_t)
        st6 = p_sm.tile([128, 6], F32, name=f"st1_{qb}", tag="st6")
        nc.vector.bn_stats(st6[:], r_t[:])
        nc.vector.bn_aggr(mv1_all[:, qb, :], st6[:])
        if qb == QBS - 1:
            _ln1_flush(0, QBS)

    # (deferred LN1 halves are emitted inside the qb loop; see _ln1_flush)

    # ---- phase 4: FFN + residual + LN2 ----------------------------------
    # FFN2 emits the output in natural [tok, D] layout directly:
    # fa[tb] += rr[ffj][:, tb-block].T @ W2[ffj]  (accumulated over ffj)
    for t in range(2):
        fa = [
            p_fa.tile([128, 512], F32, name=f"fa{t}_{j}", tag="facc") for j in range(4)
        ]
        for ffj in range(FF // 128):
            w1t = p_w1.tile([128, 4, 128], BF16, name=f"w1_{t}_{ffj}", tag="w1")
            nc.sync.dma_start(
                w1t[:],
                dram["W1"].rearrange("(c p) (f n) -> f p c n", p=128, n=128)[ffj],
            )
            w2t = w2t_all[ffj]
            f1_ps = p_mm.tile([128, 512], F32, name=f"f1_{t}_{ffj}", tag="mm")
            for c in range(4):
                nc.tensor.matmul(
                    f1_ps[:],
                    w1t[:, c, :],
                    hT[c][:, t * 512 : (t + 1) * 512],
                    start=(c == 0),
                    stop=(c == 3),
                )
            rr = p_ffr.tile([128, 512], BF16, name=f"rr{t}_{ffj}", tag="ffr")
            nc.scalar.activation(
                rr[:], f1_ps[:], AF.Relu, bias=b1_sb[:, ffj : ffj + 1]
            )
            for tb in range(4):
                nc.tensor.matmul(
                    fa[tb][:],
                    rr[:, tb * 128 : (tb + 1) * 128],
                    w2t[:],
                    start=(ffj == 0),
                    stop=(ffj == FF // 128 - 1),
                )
        for tb in range(4):
            qb = t * 4 + tb
            r2 = p_r2.tile([128, D], F32, name=f"r2_{qb}", tag="r2")
            nc.vector.tensor_tensor(r2[:], fa[tb][:], h_tiles[qb][:], op=OP.add)
            if not trivial_b2:
                nc.vector.tensor_add(r2[:], r2[:], b2b_sb[:])
            o_t = p_out.tile([128, D], F32, name=f"o{qb}", tag="out")
            layer_norm(r2, o_t, "ln2", qb)
            nc.sync.dma_start(out[qb * 128 : (qb + 1) * 128, :], o_t[:])


# ---------------------------------------------------------------------------
_PROG_CACHE = {}


def _get_program(trivial_ln: bool, trivial_b2: bool = True):
    key = (trivial_ln, trivial_b2)
    if key not in _PROG_CACHE:
        _PROG_CACHE[key] = _build_program(trivial_ln, trivial_b2)
    return _PROG_CACHE[key]


def _host_masks(window: int, chunk: int):
    """Multiplicative masks in block-transposed layout: tile[kk, kb*128+qq] =
    1.0 iff key (kb*128+kk) is visible to query qq of the block."""
    qq = np.arange(128)[:, None]
    kk = np.arange(SW)[None, :]
    band = (np.abs(kk - qq - PAD) <= window).astype(np.float32)
    m_first = band.copy()
    m_last = band.copy()
    if chunk == 0:
        m_first[:, :PAD] = 0.0
    if chunk == (L // T) - 1:
        m_last[:, SW - PAD :] = 0.0

    def blockT(m):
        out = np.empty((128, SW), np.float32)
        for kb in range(3):
            out[:, kb * 128 : (kb + 1) * 128] = m[:, kb * 128 : (kb + 1) * 128].T
        return out

    return np.stack([blockT(m_first), blockT(band), blockT(m_last)]).astype(
        ml_dtypes.bfloat16
    )


def kernel(**inputs):
    x = np.ascontiguousarray(np.asarray(inputs["x"], dtype=np.float32))
    window = int(np.asarray(inputs["window"]))
    assert 0 <= window <= PAD, f"window {window} > {PAD} not supported"
    Wq = np.ascontiguousarray(np.asarray(inputs["Wq"], np.float32))
    Wk = np.ascontiguousarray(np.asarray(inputs["Wk"], np.float32))
    Wv = np.ascontiguousarray(np.asarray(inputs["Wv"], np.float32))
    Wo = np.ascontiguousarray(np.asarray(inputs["Wo"], np.float32))
    W1 = np.ascontiguousarray(np.asarray(inputs["W1"], np.float32))
    W2 = np.ascontiguousarray(np.asarray(inputs["W2"], np.float32))
    bq = np.asarray(inputs["bq"], np.float32)
    bk = np.asarray(inputs["bk"], np.float32)
    bv = np.asarray(inputs["bv"], np.float32)
    bo = np.asarray(inputs["bo"], np.float32)
    b1 = np.asarray(inputs["b1"], np.float32)
    b2 = np.asarray(inputs["b2"], np.float32)
    g1 = np.asarray(inputs["ln1_g"], np.float32)
    be1 = np.asarray(inputs["ln1_b"], np.float32)
    g2 = np.asarray(inputs["ln2_g"], np.float32)
    be2 = np.asarray(inputs["ln2_b"], np.float32)

    trivial_ln = (
        np.all(g1 == 1.0) and np.all(be1 == 0.0)
        and np.all(g2 == 1.0) and np.all(be2 == 0.0)
    )
    trivial_b2 = bool(np.all(b2 == 0.0))
    nc = _get_program(bool(trivial_ln), trivial_b2)

    bo_p = (bv @ Wo + bo).astype(np.float32)
    ident = np.eye(128, dtype=np.float32)
    chunks = L // T  # seq chunks per batch element (4)

    in_maps = []
    for c in range(CORES):
        b, j = divmod(c, chunks)
        s = j * T
        xe = np.zeros((EXT, D), np.float32)
        lo, hi = max(0, s - PAD), min(L, s + T + PAD)
        xe[lo - (s - PAD) : hi - (s - PAD)] = x[b, lo:hi]
        vones = np.zeros((128, H, 4), ml_dtypes.bfloat16)
        vones[:, :, 0] = 1.0
        m = {
            "x_ext": xe,
            "vones": vones,
            "identb": np.eye(128, dtype=ml_dtypes.bfloat16),
            "xr": (x[b, s : s + T] + bo_p).astype(np.float32),
            "masks": _host_masks(window, j),
            "ident": ident,
            "Wq": Wq, "Wk": Wk, "Wv": Wv,
            "Wo": Wo.astype(ml_dtypes.bfloat16),
            "W1": W1.astype(ml_dtypes.bfloat16),
            "W2": W2.astype(ml_dtypes.bfloat16),
            "bq": bq, "bk": bk, "b1": b1, "b2": b2,
        }
        if not trivial_ln:
            m.update({"ln1_g": g1, "ln1_b": be1, "ln2_g": g2, "ln2_b": be2})
        in_maps.append(m)

    if not getattr(nc, "_waits_split", False):
        _split_excess_waits(nc)
        nc._waits_split = True
    res = run_bass_kernel_spmd(nc, in_maps, core_ids=list(range(CORES)))
    outs = np.stack([res.results[c]["out"] for c in range(CORES)])
    return outs.reshape(B, L, D)


# revision 27
# speedup vs baseline: 1.2423x; 1.2134x over previous
"""Trainium2 Bass kernel: Longformer block (banded self-attention + FFN).

Full inputs come in, full output goes out. Internally shards over 8
NeuronCores: core c handles batch c//4, sequence chunk c%4 (1024 tokens),
with a 128-token halo on each side for the local attention window.
Everything else (QKV, banded softmax, Wo, LN1, FFN, LN2) is computed
per-core with no collectives.
"""

import sys
import types
import numpy as np
import ml_dtypes
from contextlib import ExitStack

import concourse.bass as bass
import concourse.mybir as mybir
import concourse.tile as tile
from concourse.vector_clock import ScopedClock
from concourse.bass_utils import run_bass_kernel_spmd

F32 = mybir.dt.float32
F32R = mybir.dt.float32r
BF16 = mybir.dt.bfloat16
AF = mybir.ActivationFunctionType
OP = mybir.AluOpType

# problem shape (hardcoded)
B, L, D, H, HD, FF = 2, 4096, 512, 8, 64, 2048
CORES = 8
T = (B * L) // CORES          # 1024 tokens per core
PAD = 128                     # halo
EXT = T + 2 * PAD             # 1280
QBS = T // 128                # 8 query blocks / core
SW = 3 * 128                  # 384-wide key strip per query block
EPS = 1e-5
NEG = -1e30

# ---------------------------------------------------------------------------
# The final Tile drain on this walrus build only supports ONE sync-wait per
# CTRL instruction; stock TileContext funnels every live semaphore into a
# single drain.  Split the waits across a chain of drains instead.
_MAX_DRAIN_WAITS = 1


def _patched_drain_and_barrier(self, tick_clock, wait_clock):
    nc = self.nc
    drain_inst = nc.sync.drain()
    wait_clock.add_sem_waits(
        drain_inst.ins, ScopedClock({None: tick_clock.global_clock})
    )
    mi = drain_inst.ins
    if mi.sync_info is not None:
        waits = list(mi.sync_info.on_wait or [])
        if len(waits) > _MAX_DRAIN_WAITS:
            mi.sync_info = mybir.SyncInfo(
                on_wait=waits[:_MAX_DRAIN_WAITS],
                on_update=list(mi.sync_info.on_update or []),
            )
            # spread remaining waits across engines so they resolve in
            # parallel; the all_engine_barrier below orders everyone.
            engines = [nc.vector, nc.scalar, nc.tensor, nc.gpsimd, nc.sync]
            for i, w in enumerate(waits[_MAX_DRAIN_WAITS:]):
                eng = engines[i % len(engines)]
                n2 = eng.nop()
                n2.ins.sync_info = mybir.SyncInfo(on_wait=[w], on_update=[])
    nc.all_engine_barrier()
    assert self.sems is not None
    popped = nc._tile_sem_poison_stack.pop()
    assert popped is self._sem_poison
    nc.clear_and_free_semaphores(list(self.sems.allocated().values()))
    nc.all_engine_barrier()


tile.TileContext._drain_and_barrier = _patched_drain_and_barrier

_MAX_INST_WAITS = 1
_nop_counter = [0]


def _split_excess_waits(nc, limit=_MAX_INST_WAITS):
    """walrus on this image accepts only `limit` sync-waits per instruction;
    move excess waits onto injected same-engine NoOps placed just before."""
    for f in nc.m.functions:
        for bb in f.blocks:
            out = []
            changed = False
            for inst in bb.instructions:
                si = inst.sync_info
                waits = list(si.on_wait) if (si is not None and si.on_wait) else []
                if len(waits) > limit:
                    changed = True
                    extra = waits[: len(waits) - limit]
                    keep = waits[len(waits) - limit :]
                    for i in range(0, len(extra), max(1, limit)):
                        _nop_counter[0] += 1
                        nop = mybir.InstNoOp(name=f"nopw-{_nop_counter[0]}", ins=[], outs=[])
                        nop.engine = inst.engine
                        nop.sync_info = mybir.SyncInfo(
                            on_wait=extra[i : i + max(1, limit)], on_update=[]
                        )
                        out.append(nop)
                    inst.sync_info = mybir.SyncInfo(
                        on_wait=keep, on_update=list(si.on_update or [])
                    )
                out.append(inst)
            if changed:
                bb.instructions = out


# ---------------------------------------------------------------------------
def _build_program(trivial_ln: bool, trivial_b2: bool):
    nc = bass.Bass("TRN2", target_bir_lowering=False, debug=False)

    dram = {}
    def din(name, shape):
        dram[name] = nc.dram_tensor(name, shape, F32, kind="ExternalInput").ap()
        return dram[name]

    xr = din("xr", [T, D])              # x_core + (bv@Wo + bo), residual input
    def dinr(name, shape):
        dram[name] = nc.dram_tensor(name, shape, F32R, kind="ExternalInput").ap()
        return dram[name]

    ident = dinr("ident", [128, 128])

    def dinb(name, shape):
        dram[name] = nc.dram_tensor(name, shape, BF16, kind="ExternalInput").ap()
        return dram[name]

    identb = dinb("identb", [128, 128])
    x_ext = dinb("x_ext", [EXT, D])
    Wod = dinb("Wo", [D, D])
    W1d = dinb("W1", [D, FF]); W2d = dinb("W2", [FF, D])
    vones = dinb("vones", [128, H, 4])
    masks = dinb("masks", [3, 128, SW])

    Wqd = dinb("Wq", [D, D]); Wkd = dinb("Wk", [D, D]); Wvd = dinb("Wv", [D, D])
    bqd = din("bq", [D]); bkd = din("bk", [D])
    b1d = din("b1", [FF]); b2d = din("b2", [D])
    if not trivial_ln:
        g1d = din("ln1_g", [D]); be1d = din("ln1_b", [D])
        g2d = din("ln2_g", [D]); be2d = din("ln2_b", [D])
    out = nc.dram_tensor("out", [T, D], F32, kind="ExternalOutput").ap()

    with tile.TileContext(nc) as tc:
        with ExitStack() as ctx:
            _build_body(ctx, tc, dram, out, trivial_ln, trivial_b2)
    return nc


def _build_body(ctx, tc, dram, out, trivial_ln, trivial_b2):
    nc = tc.nc

    def pool(name, bufs, space="SBUF"):
        return ctx.enter_context(tc.tile_pool(name=name, bufs=bufs, space=space))

    p_const = pool("const", 1)
    p_mm = pool("mm", 2, "PSUM")
    p_fa = pool("facc", 4, "PSUM")
    p_actT = pool("actT", 4)     # xT chunks, slots later recycled for hT
    p_kT = pool("kT", 4)
    p_v = pool("v", 10)
    p_qT = pool("qT", 4)
    p_xn = pool("xn", 2)
    p_w = pool("w", 6)
    p_wo = pool("wo", 4)
    p_w1 = pool("w1", 10)
    p_w2 = pool("w2", 16)
    p_xr = pool("xr", 2)
    p_probs = pool("probs", 5)
    p_pT = pool("pT", 3)
    p_attn = pool("attn", 2)
    p_aT = pool("aT", 2)
    p_r = pool("r", 8)
    p_ffr = pool("ffr", 3)
    p_r2 = pool("r2", 3)
    p_out = pool("out", 3)
    p_sm = pool("sm", 4)

    # ---- constants
    ident_sb = p_const.tile([128, 128], F32R, name="ident", tag="ident")
    nc.sync.dma_start(ident_sb[:], dram["ident"][:])
    identb_sb = p_const.tile([128, 128], BF16, name="identb", tag="identb")
    nc.sync.dma_start(identb_sb[:], dram["identb"][:])
    maskT_sb = []
    for m in range(3):
        mt = p_const.tile([128, SW], BF16, name=f"maskT{m}", tag=f"maskT{m}")
        maskT_sb.append(mt)
    bq_sb = p_const.tile([128, 4], F32, name="bq", tag="bq")
    nc.sync.dma_start(bq_sb[:], dram["bq"].rearrange("(j p) -> p j", p=128))
    bk_sb = p_const.tile([128, 4], F32, name="bk", tag="bk")
    nc.sync.dma_start(bk_sb[:], dram["bk"].rearrange("(j p) -> p j", p=128))
    b1_sb = p_const.tile([128, 16], F32, name="b1", tag="b1")
    nc.sync.dma_start(b1_sb[:], dram["b1"].rearrange("(j p) -> p j", p=128))
    if not trivial_b2:
        b2b_sb = p_const.tile([128, D], F32, name="b2b", tag="b2b")
        nc.sync.dma_start(b2b_sb[0:1, :], dram["b2"].rearrange("d -> 1 d"))
        nc.gpsimd.partition_broadcast(b2b_sb[:], b2b_sb[0:1, :])
    eps_sb = p_const.tile([128, 1], F32, name="eps", tag="eps")
    nc.vector.memset(eps_sb[:], EPS)
    mv1_all = p_const.tile([128, QBS, 2], F32, name="mv1all", tag="mv1all")

    gb = {}
    if not trivial_ln:
        for nm in ("ln1_g", "ln1_b", "ln2_g", "ln2_b"):
            tile_ = p_const.tile([128, D], F32, name=nm, tag=nm)
            nc.sync.dma_start(
                tile_[0:1, :], dram[nm].rearrange("d -> 1 d")
            )
            nc.gpsimd.partition_broadcast(tile_[:], tile_[0:1, :])
            gb[nm] = tile_

    # ---- phase 1: load x, build xT (transposed activations) -------------
    xT = [p_actT.tile([128, EXT], BF16, name=f"xT{c}", tag="actT") for c in range(4)]
    for tb in range(EXT // 128):
        xn = p_xn.tile([128, D], BF16, name=f"xn{tb}", tag="xn")
        nc.sync.dma_start(xn[:], dram["x_ext"][tb * 128 : (tb + 1) * 128, :])
        ps = p_mm.tile([128, 1024], BF16, name=f"xTp{tb}", tag="mmb", bufs=2, padded_shape=[128, 1024])
        for j in range(4):
            nc.tensor.matmul(
                ps[:, j * 128 : (j + 1) * 128],
                xn[:, j * 128 : (j + 1) * 128],
                identb_sb[:],
                is_transpose=True,
                start=(j == 0),
                stop=(j == 3),
            )
        for j in range(4):
            nc.vector.tensor_copy(
                xT[j][:, tb * 128 : (tb + 1) * 128], ps[:, j * 128 : (j + 1) * 128]
            )

    # ---- phase 2: QKV projections ---------------------------------------
    kT = [p_kT.tile([128, EXT], BF16, name=f"kT{c}", tag="kT") for c in range(4)]
    qT = [p_qT.tile([128, T], BF16, name=f"qT{c}", tag="qT") for c in range(4)]
    V = [p_v.tile([128, H, HD + 4], BF16, name=f"v{tb}", tag="v") for tb in range(EXT // 128)]

    def lin_T(dst, wd, bias_sb, tok0, ntok, wname):
        """dst[j] [128, ntok] = (x @ W + b).T ; x cols [tok0, tok0+ntok) of xT."""
        wt = []
        for c in range(4):
            w = p_w.tile([128, D], BF16, name=f"{wname}{c}", tag="w")
            nc.sync.dma_start(w[:], wd[c * 128 : (c + 1) * 128, :])
            wt.append(w)
        for j in range(4):
            for t0 in range(0, ntok, 512):
                tw = min(512, ntok - t0)
                ps = p_mm.tile([128, 512], F32, name=f"{wname}p{j}_{t0}", tag="mm")
                for c in range(4):
                    nc.tensor.matmul(
                        ps[:, :tw],
                        wt[c][:, j * 128 : (j + 1) * 128],
                        xT[c][:, tok0 + t0 : tok0 + t0 + tw],
                        start=(c == 0),
                        stop=(c == 3),
                    )
                if bias_sb is not None:
                    nc.vector.tensor_scalar_add(
                        dst[j][:, t0 : t0 + tw], ps[:, :tw], bias_sb[:, j : j + 1]
                    )
                else:
                    nc.vector.tensor_copy(dst[j][:, t0 : t0 + tw], ps[:, :tw])

    lin_T(kT, dram["Wk"], bk_sb, 0, EXT, "wk")
    lin_T(qT, dram["Wq"], bq_sb, PAD, T, "wq")

    # V in natural layout [tok, dv]; bv folded into xr on the host.
    wvt = []
    for c in range(4):
        w = p_w.tile([128, D], BF16, name=f"wv{c}", tag="w")
        nc.sync.dma_start(w[:], dram["Wv"][c * 128 : (c + 1) * 128, :])
        wvt.append(w)
    for tb in range(EXT // 128):
        ps = p_mm.tile([128, 512], F32, name=f"vp{tb}", tag="mm")
        for c in range(4):
            nc.tensor.matmul(
                ps[:],
                xT[c][:, tb * 128 : (tb + 1) * 128],
                wvt[c][:],
                start=(c == 0),
                stop=(c == 3),
            )
        nc.vector.tensor_copy(
            V[tb][:, :, 0:HD], ps[:].rearrange("p (h d) -> p h d", d=HD)
        )
        nc.sync.dma_start(V[tb][:, :, HD : HD + 4], dram["vones"][:])

    for m in range(3):
        nc.sync.dma_start(maskT_sb[m][:], dram["masks"][m])
    # Wo, resident through attention
    wot = []
    for j in range(4):
        w = p_wo.tile([128, D], BF16, name=f"wo{j}", tag="wo")
        nc.sync.dma_start(w[:], dram["Wo"][j * 128 : (j + 1) * 128, :])
        wot.append(w)
    # W2 resident for both FFN token chunks
    w2t_all = []
    for ffj in range(FF // 128):
        w = p_w2.tile([128, D], BF16, name=f"w2r_{ffj}", tag="w2")
        nc.sync.dma_start(w[:], dram["W2"][ffj * 128 : (ffj + 1) * 128, :])
        w2t_all.append(w)

    # ---- phase 3: banded attention + Wo + residual + LN1 ----------------
    hT = [p_actT.tile([128, T], BF16, name=f"hT{c}", tag="actT") for c in range(4)]
    h_tiles = []

    def layer_norm(r_t, dst, g_key, qb):
        st6 = p_sm.tile([128, 6], F32, name=f"st{g_key}{qb}", tag="st6")
        nc.vector.bn_stats(st6[:], r_t[:])
        mv = p_sm.tile([128, 2], F32, name=f"mv{g_key}{qb}", tag="mv")
        nc.vector.bn_aggr(mv[:], st6[:])
        std = p_sm.tile([128, 1], F32, name=f"sd{g_key}{qb}", tag="std")
        nc.scalar.activation(std[:], mv[:, 1:2], AF.Sqrt, bias=eps_sb[:])
        rs = p_sm.tile([128, 1], F32, name=f"rs{g_key}{qb}", tag="rs")
        nc.vector.reciprocal(rs[:], std[:])
        nm = p_sm.tile([128, 1], F32, name=f"nm{g_key}{qb}", tag="nm")
        nc.vector.scalar_tensor_tensor(
            nm[:], mv[:, 0:1], -1.0, rs[:], op0=OP.mult, op1=OP.mult
        )
        nc.scalar.activation(dst[:], r_t[:], AF.Identity, bias=nm[:], scale=rs[:])
        if not trivial_ln:
            nc.vector.tensor_mul(dst[:], dst[:], gb[f"{g_key}_g"][:])
            nc.vector.tensor_add(dst[:], dst[:], gb[f"{g_key}_b"][:])

    r_tiles = []

    def _ln1_flush(lo, hi):
        """LN1 sqrt/recip/apply + h transposes for q-blocks [lo, hi)."""
        std4 = p_sm.tile([128, hi - lo], F32, name=f"std{lo}", tag="std8")
        nc.scalar.activation(std4[:], mv1_all[:, lo:hi, 1], AF.Sqrt, bias=eps_sb[:])
        rec4 = p_sm.tile([128, hi - lo], F32, name=f"recb{lo}", tag="rec8")
        nc.vector.reciprocal(rec4[:], std4[:])
        nm4 = p_sm.tile([128, hi - lo], F32, name=f"nmb{lo}", tag="nm8")
        nc.vector.scalar_tensor_tensor(
            nm4[:], mv1_all[:, lo:hi, 0], -1.0, rec4[:], op0=OP.mult, op1=OP.mult
        )
        for qb2 in range(lo, hi):
            h_t = r_tiles[qb2]
            nc.scalar.activation(
                h_t[:], r_tiles[qb2][:], AF.Identity,
                bias=nm4[:, qb2 - lo : qb2 - lo + 1],
                scale=rec4[:, qb2 - lo : qb2 - lo + 1],
            )
            if not trivial_ln:
                nc.vector.tensor_mul(h_t[:], h_t[:], gb["ln1_g"][:])
                nc.vector.tensor_add(h_t[:], h_t[:], gb["ln1_b"][:])
            h_tiles.append(h_t)
            h_bf = p_aT.tile([128, D], BF16, name=f"hbf{qb2}", tag="hbf", bufs=2)
            nc.vector.tensor_copy(h_bf[:], h_t[:])
            hT_ps = p_mm.tile([128, 1024], BF16, name=f"hTp{qb2}", tag="mmb", bufs=2, padded_shape=[128, 1024])
            for j in range(4):
                nc.tensor.matmul(
                    hT_ps[:, j * 128 : (j + 1) * 128],
                    h_bf[:, j * 128 : (j + 1) * 128],
                    identb_sb[:],
                    is_transpose=True,
                    start=(j == 0),
                    stop=(j == 3),
                )
            for j in range(4):
                nc.vector.tensor_copy(
                    hT[j][:, qb2 * 128 : (qb2 + 1) * 128],
                    hT_ps[:, j * 128 : (j + 1) * 128],
                )

    for qb in range(QBS):
        xr_t = p_xr.tile([128, D], F32, name=f"xr{qb}", tag="xr")
        nc.sync.dma_start(xr_t[:], dram["xr"][qb * 128 : (qb + 1) * 128, :])
        attn_sb = p_attn.tile([128, D], BF16, name=f"attn{qb}", tag="attn")
        rec = p_sm.tile([128, H], F32, name=f"rec{qb}", tag="rec")
        mk = maskT_sb[0] if qb == 0 else (
            maskT_sb[2] if qb == QBS - 1 else maskT_sb[1])

        sc_ps = [None] * H

        def emit_scores(h):
            cch, roff = h // 2, (h % 2) * 64
            ps = p_fa.tile([128, 512], F32, name=f"sc{qb}_{h}", tag="facc")
            nc.tensor.matmul(
                ps[:, :SW],
                qT[cch][roff : roff + 64, qb * 128 : (qb + 1) * 128],
                kT[cch][roff : roff + 64, qb * 128 : qb * 128 + SW],
                start=True,
                stop=True,
            )
            sc_ps[h] = ps

        for h0 in range(4):
            emit_scores(h0)

        prs = [None] * H
        pT_pss = [None] * H
        pTs = [None] * H

        def emit_exp(h):
            pr = p_probs.tile([128, SW], BF16, name=f"pr{qb}_{h}", tag="probs")
            nc.scalar.activation(
                pr[:], sc_ps[h][:, :SW], AF.Exp, scale=float(1.0 / np.sqrt(HD))
            )
            prs[h] = pr

        def emit_T(h):
            pT_ps = p_mm.tile([128, 1024], BF16, name=f"pT{qb}_{h}", tag="mmb", bufs=2, padded_shape=[128, 1024])
            for kb in range(3):
                nc.tensor.matmul(
                    pT_ps[:, kb * 128 : (kb + 1) * 128],
                    prs[h][:, kb * 128 : (kb + 1) * 128],
                    identb_sb[:],
                    is_transpose=True,
                    start=(kb == 0),
                    stop=(kb == 2),
                )
            pT_pss[h] = pT_ps

        emit_exp(0)
        emit_exp(1)
        emit_T(0)
        for h in range(H):
            if h + 2 < H:
                emit_exp(h + 2)
            if h + 1 < H:
                emit_T(h + 1)
            pT = p_pT.tile([128, SW], BF16, name=f"pTs{qb}_{h}", tag="pT")
            nc.vector.tensor_tensor(pT[:], pT_pss[h][:, :SW], mk[:], op=OP.mult)
            if h + 4 < H:
                emit_scores(h + 4)
            at_ps = p_mm.tile([128, 512], F32, name=f"at{qb}_{h}", tag="mm")
            for kb in range(3):
                nc.tensor.matmul(
                    at_ps[:, : HD + 4],
                    pT[:, kb * 128 : (kb + 1) * 128],
                    V[qb + kb][:, h, :],
                    start=(kb == 0),
                    stop=(kb == 2),
                )
            nc.vector.reciprocal(rec[:, h : h + 1], at_ps[:, HD : HD + 1])
            nc.vector.tensor_scalar_mul(
                attn_sb[:, h * HD : (h + 1) * HD], at_ps[:, :HD], rec[:, h : h + 1]
            )
        # attn -> attn_T -> @Wo -> + xr ; LN1 stats only (apply deferred)
        aT_ps = p_mm.tile([128, 1024], BF16, name=f"aTp{qb}", tag="mmb", bufs=2, padded_shape=[128, 1024])
        for j in range(4):
            nc.tensor.matmul(
                aT_ps[:, j * 128 : (j + 1) * 128],
                attn_sb[:, j * 128 : (j + 1) * 128],
                identb_sb[:],
                is_transpose=True,
                start=(j == 0),
                stop=(j == 3),
            )
        aT = p_aT.tile([128, D], BF16, name=f"aT{qb}", tag="aT")
        nc.vector.tensor_copy(aT[:], aT_ps[:, :D])
        wo_ps = p_mm.tile([128, 512], F32, name=f"wop{qb}", tag="mm")
        for j in range(4):
            nc.tensor.matmul(
                wo_ps[:],
                aT[:, j * 128 : (j + 1) * 128],
                wot[j][:],
                start=(j == 0),
                stop=(j == 3),
            )
        r_t = p_r.tile([128, D], F32R, name=f"r{qb}", tag="r")
        nc.vector.tensor_tensor(r_t[:], wo_ps[:], xr_t[:], op=OP.add)
        r_tiles.append(r_t)
        st6 = p_sm.tile([128, 6], F32, name=f"st1_{qb}", tag="st6")
        nc.vector.bn_stats(st6[:], r_t[:])
        nc.vector.bn_aggr(mv1_all[:, qb, :], st6[:])
        if qb == QBS - 1:
            _ln1_flush(0, QBS)

    # (deferred LN1 halves are emitted inside the qb loop; see _ln1_flush)

    # ---- phase 4: FFN + residual + LN2 ----------------------------------
    # FFN2 emits the output in natural [tok, D] layout directly:
    # fa[tb] += rr[ffj][:, tb-block].T @ W2[ffj]  (accumulated over ffj)
    for t in range(2):
        fa = [
            p_fa.tile([128, 512], F32, name=f"fa{t}_{j}", tag="facc") for j in range(4)
        ]
        for ffj in range(FF // 128):
            w1t = p_w1.tile([128, 4, 128], BF16, name=f"w1_{t}_{ffj}", tag="w1")
            nc.sync.dma_start(
                w1t[:],
                dram["W1"].rearrange("(c p) (f n) -> f p c n", p=128, n=128)[ffj],
            )
            w2t = w2t_all[ffj]
            f1_ps = p_mm.tile([128, 512], F32, name=f"f1_{t}_{ffj}", tag="mm")
            for c in range(4):
                nc.tensor.matmul(
                    f1_ps[:],
                    w1t[:, c, :],
                    hT[c][:, t * 512 : (t + 1) * 512],
                    start=(c == 0),
                    stop=(c == 3),
                )
            rr = p_ffr.tile([128, 512], BF16, name=f"rr{t}_{ffj}", tag="ffr")
            nc.scalar.activation(
                rr[:], f1_ps[:], AF.Relu, bias=b1_sb[:, ffj : ffj + 1]
            )
            for tb in range(4):
                nc.tensor.matmul(
                    fa[tb][:],
                    rr[:, tb * 128 : (tb + 1) * 128],
                    w2t[:],
                    start=(ffj == 0),
                    stop=(ffj == FF // 128 - 1),
                )
        for tb in range(4):
            qb = t * 4 + tb
            r2 = p_r2.tile([128, D], F32, name=f"r2_{qb}", tag="r2")
            nc.vector.tensor_tensor(r2[:], fa[tb][:], h_tiles[qb][:], op=OP.add)
            if not trivial_b2:
                nc.vector.tensor_add(r2[:], r2[:], b2b_sb[:])
            o_t = p_out.tile([128, D], F32, name=f"o{qb}", tag="out")
            layer_norm(r2, o_t, "ln2", qb)
            nc.sync.dma_start(out[qb * 128 : (qb + 1) * 128, :], o_t[:])


# ---------------------------------------------------------------------------
_PROG_CACHE = {}


def _get_program(trivial_ln: bool, trivial_b2: bool = True):
    key = (trivial_ln, trivial_b2)
    if key not in _PROG_CACHE:
        _PROG_CACHE[key] = _build_program(trivial_ln, trivial_b2)
    return _PROG_CACHE[key]


def _host_masks(window: int, chunk: int):
    """Multiplicative masks in block-transposed layout: tile[kk, kb*128+qq] =
    1.0 iff key (kb*128+kk) is visible to query qq of the block."""
    qq = np.arange(128)[:, None]
    kk = np.arange(SW)[None, :]
    band = (np.abs(kk - qq - PAD) <= window).astype(np.float32)
    m_first = band.copy()
    m_last = band.copy()
    if chunk == 0:
        m_first[:, :PAD] = 0.0
    if chunk == (L // T) - 1:
        m_last[:, SW - PAD :] = 0.0

    def blockT(m):
        out = np.empty((128, SW), np.float32)
        for kb in range(3):
            out[:, kb * 128 : (kb + 1) * 128] = m[:, kb * 128 : (kb + 1) * 128].T
        return out

    return np.stack([blockT(m_first), blockT(band), blockT(m_last)]).astype(
        ml_dtypes.bfloat16
    )


def kernel(**inputs):
    x = np.ascontiguousarray(np.asarray(inputs["x"], dtype=np.float32))
    window = int(np.asarray(inputs["window"]))
    assert 0 <= window <= PAD, f"window {window} > {PAD} not supported"
    Wq = np.ascontiguousarray(np.asarray(inputs["Wq"], np.float32))
    Wk = np.ascontiguousarray(np.asarray(inputs["Wk"], np.float32))
    Wv = np.ascontiguousarray(np.asarray(inputs["Wv"], np.float32))
    Wo = np.ascontiguousarray(np.asarray(inputs["Wo"], np.float32))
    W1 = np.ascontiguousarray(np.asarray(inputs["W1"], np.float32))
    W2 = np.ascontiguousarray(np.asarray(inputs["W2"], np.float32))
    bq = np.asarray(inputs["bq"], np.float32)
    bk = np.asarray(inputs["bk"], np.float32)
    bv = np.asarray(inputs["bv"], np.float32)
    bo = np.asarray(inputs["bo"], np.float32)
    b1 = np.asarray(inputs["b1"], np.float32)
    b2 = np.asarray(inputs["b2"], np.float32)
    g1 = np.asarray(inputs["ln1_g"], np.float32)
    be1 = np.asarray(inputs["ln1_b"], np.float32)
    g2 = np.asarray(inputs["ln2_g"], np.float32)
    be2 = np.asarray(inputs["ln2_b"], np.float32)

    trivial_ln = (
        np.all(g1 == 1.0) and np.all(be1 == 0.0)
        and np.all(g2 == 1.0) and np.all(be2 == 0.0)
    )
    trivial_b2 = bool(np.all(b2 == 0.0))
    nc = _get_program(bool(trivial_ln), trivial_b2)

    bo_p = (bv @ Wo + bo).astype(np.float32)
    ident = np.eye(128, dtype=np.float32)
    chunks = L // T  # seq chunks per batch element (4)

    in_maps = []
    for c in range(CORES):
        b, j = divmod(c, chunks)
        s = j * T
        xe = np.zeros((EXT, D), np.float32)
        lo, hi = max(0, s - PAD), min(L, s + T + PAD)
        xe[lo - (s - PAD) : hi - (s - PAD)] = x[b, lo:hi]
        vones = np.zeros((128, H, 4), ml_dtypes.bfloat16)
        vones[:, :, 0] = 1.0
        m = {
            "x_ext": xe.astype(ml_dtypes.bfloat16),
            "vones": vones,
            "identb": np.eye(128, dtype=ml_dtypes.bfloat16),
            "xr": (x[b, s : s + T] + bo_p).astype(np.float32),
            "masks": _host_masks(window, j),
            "ident": ident,
            "Wq": Wq.astype(ml_dtypes.bfloat16),
            "Wk": Wk.astype(ml_dtypes.bfloat16),
            "Wv": Wv.astype(ml_dtypes.bfloat16),
            "Wo": Wo.astype(ml_dtypes.bfloat16),
            "W1": W1.astype(ml_dtypes.bfloat16),
            "W2": W2.astype(ml_dtypes.bfloat16),
            "bq": bq, "bk": bk, "b1": b1, "b2": b2,
        }
        if not trivial_ln:
            m.update({"ln1_g": g1, "ln1_b": be1, "ln2_g": g2, "ln2_b": be2})
        in_maps.append(m)

    if not getattr(nc, "_waits_split", False):
        _split_excess_waits(nc)
        nc._waits_split = True
    res = run_bass_kernel_spmd(nc, in_maps, core_ids=list(range(CORES)))
    outs = np.stack([res.results[c]["out"] for c in range(CORES)])
    return outs.reshape(B, L, D)
